# revision 14
# baseline (speedup 1.0000x reference)
"""2-layer GCN (PyG GCNConv semantics) on 8 Trainium2 NeuronCores.

Strategy (per the node-sharding hint):
  - Nodes are sharded contiguously across the 8 cores (dst-ownership).
  - Layer tables T1 = dis*(x@W1)  [N,64] f32 and T2 = (dis*relu(out1))@W2
    [N,48] f32 are computed shard-locally and AllGather'd so every core can
    gather any source row.
  - Per core, edges are grouped by destination into fixed-K windows of 128
    degree-sorted nodes; messages are fetched with 128-row indirect DMA
    gathers (one index per partition), summed with a tensor_tensor add tree,
    scaled by dis[dst], biased, (relu'd), and scattered back to node-id rows
    with an indirect DMA scatter.
  - dis[src] is folded into the tables; dis[dst] is a per-partition scalar.

kernel(**inputs) takes the FULL inputs and returns the FULL [N,40] output.
"""

import numpy as np
import ml_dtypes

import concourse.bass as bass
import concourse.bacc as bacc
import concourse.tile as tile
import concourse.mybir as mybir
from concourse import bass_utils

F32 = mybir.dt.float32
BF16 = mybir.dt.bfloat16
I32 = mybir.dt.int32

NCORES = 8
GHOST = 1 << 20  # scatter index sentinel, skipped via bounds_check


def _round_up(x, m):
    return ((x + m - 1) // m) * m


def _prep(x, edge_index, W1, b1, W2, b2):
    """Host-side graph partitioning + metadata packing (numpy only)."""
    N, IN_DIM = x.shape
    HID = W1.shape[1]
    OUT = W2.shape[1]
    OUTP = _round_up(OUT, 8)  # padded table-2 width (48 for OUT=40)
    assert N % NCORES == 0
    SHARD = N // NCORES  # nodes per core
    ROWS = SHARD + 1  # per-shard table rows incl zero row
    NT = _round_up(SHARD, 128) // 128  # 128-node tiles per shard
    SHARD_PAD = NT * 128

    src = edge_index[0].astype(np.int64)
    dst = edge_index[1].astype(np.int64)
    # self-loops
    loops = np.arange(N, dtype=np.int64)
    src = np.concatenate([src, loops])
    dst = np.concatenate([dst, loops])
    deg = np.bincount(dst, minlength=N).astype(np.float64)  # >=1 (self-loops)
    dis = (1.0 / np.sqrt(deg)).astype(np.float32)

    # global table row of node g (tables have a zero row per shard)
    def grow(g):
        return (g // SHARD) * ROWS + (g % SHARD)

    core_of = (dst // SHARD).astype(np.int64)

    # pass 1: per-core degree-sorted windows -> global K schedule
    orders = []
    degl_list = []
    for c in range(NCORES):
        m = core_of == c
        dl = (dst[m] - c * SHARD).astype(np.int64)
        degl = np.bincount(dl, minlength=SHARD)
        order = np.argsort(-degl, kind="stable").astype(np.int64)  # desc degree
        orders.append(order)
        degl_list.append(degl)
    NWIN = NT
    K = np.zeros(NWIN, dtype=np.int64)
    for c in range(NCORES):
        degl = degl_list[c]
        order = orders[c]
        for w in range(NWIN):
            nodes = order[w * 128 : (w + 1) * 128]
            if len(nodes):
                K[w] = max(K[w], degl[nodes].max() if len(nodes) else 0)
    K = np.maximum(((K + 1) // 2) * 2, 2)
    coff = np.concatenate([[0], np.cumsum(K)]).astype(np.int64)
    NCH = int(coff[-1])

    in_maps = []
    for c in range(NCORES):
        m = core_of == c
        s_c = src[m]
        d_c = dst[m]
        dl = (d_c - c * SHARD).astype(np.int64)
        order = orders[c]
        inv = np.empty(SHARD, dtype=np.int64)
        inv[order] = np.arange(SHARD)
        pos = inv[dl]  # degree-sorted position of each edge's dst
        o2 = np.argsort(pos, kind="stable")
        pos_s = pos[o2]
        src_s = s_c[o2]
        first = np.searchsorted(pos_s, pos_s, side="left")
        slot = np.arange(len(pos_s)) - first
        lane = pos_s % 128
        win = pos_s // 128
        col = coff[win] + slot
        ZROW = SHARD  # shard 0's zero row (global table row = SHARD)
        gidx = np.full((128, NCH), ZROW, dtype=np.int32)
        gidx[lane, col] = grow(src_s).astype(np.int32)

        # window metadata in degree-sorted order
        node_of = np.full((128, NWIN), -1, dtype=np.int64)
        for w in range(NWIN):
            nodes = order[w * 128 : min((w + 1) * 128, SHARD)]
            node_of[: len(nodes), w] = nodes
        real = node_of >= 0
        disw = np.zeros((128, NWIN), dtype=np.float32)
        disw[real] = dis[c * SHARD + node_of[real]]
        scat = np.full((128, NWIN), GHOST, dtype=np.int32)
        scat[real] = node_of[real].astype(np.int32)

        # id-order dis for phase 1/3 epilogues (padded tail -> 0)
        disid = np.zeros((128, NT), dtype=np.float32)
        ids = np.arange(SHARD_PAD).reshape(NT, 128).T
        okm = ids < SHARD
        disid[okm] = dis[c * SHARD + ids[okm]]

        xT = np.zeros((IN_DIM, SHARD_PAD), dtype=ml_dtypes.bfloat16)
        xT[:, :SHARD] = x[c * SHARD : (c + 1) * SHARD].T.astype(
            ml_dtypes.bfloat16
        )

        W2p = np.zeros((128, OUTP), dtype=ml_dtypes.bfloat16)
        W2p[:HID, :OUT] = W2.astype(ml_dtypes.bfloat16)

        in_maps.append(
            {
                "xT": xT,
                "gidx": gidx,
                "disw": disw,
                "scat": scat,
                "disid": disid,
                "W1": W1.astype(ml_dtypes.bfloat16),
                "W2p": W2p,
                "b1t": np.tile(np.asarray(b1, np.float32)[None, :], (128, 1)),
                "b2t": np.tile(
                    np.pad(np.asarray(b2, np.float32), (0, OUTP - OUT))[None, :],
                    (128, 1),
                ),
            }
        )

    dims = dict(
        N=N, IN_DIM=IN_DIM, HID=HID, OUT=OUT, OUTP=OUTP, SHARD=SHARD, ROWS=ROWS,
        NT=NT, SHARD_PAD=SHARD_PAD, NWIN=NWIN, NCH=NCH,
    )
    return in_maps, [int(k) for k in K], [int(v) for v in coff], dims


def _tree_reduce(nc, pool, g, K, F):
    """Sum g's [128, K, F] f32 chunks into a [128, F] tile."""
    cur = g
    n = K
    lvl = 0
    while n % 2 == 0 and n > 2:
        h = n // 2
        dst = pool.tile([128, h * F], F32, tag=f"lvl{lvl}")
        nc.vector.tensor_add(dst[:], cur[:, : h * F], cur[:, h * F : 2 * h * F])
        cur = dst
        n = h
        lvl += 1
    acc = pool.tile([128, F], F32, tag="acc")
    nc.vector.tensor_add(acc[:], cur[:, :F], cur[:, F : 2 * F])
    for i in range(2, n):
        nc.vector.tensor_add(acc[:], acc[:], cur[:, i * F : (i + 1) * F])
    return acc


def _build(K, coff, d):
    N, HID, OUTP, OUT = d["N"], d["HID"], d["OUTP"], d["OUT"]
    IN_DIM, SHARD, ROWS, NT = d["IN_DIM"], d["SHARD"], d["ROWS"], d["NT"]
    SHARD_PAD, NWIN, NCH = d["SHARD_PAD"], d["NWIN"], d["NCH"]
    HPAD = SHARD_PAD  # h_local rows (multiple of 128 for DMA transpose)

    nc = bacc.Bacc("TRN2", target_bir_lowering=False, debug=False,
                   num_devices=NCORES)
    xT = nc.dram_tensor("xT", [IN_DIM, SHARD_PAD], BF16, kind="ExternalInput")
    gidx_d = nc.dram_tensor("gidx", [128, NCH], I32, kind="ExternalInput")
    disw_d = nc.dram_tensor("disw", [128, NWIN], F32, kind="ExternalInput")
    scat_d = nc.dram_tensor("scat", [128, NWIN], I32, kind="ExternalInput")
    disid_d = nc.dram_tensor("disid", [128, NT], F32, kind="ExternalInput")
    W1_d = nc.dram_tensor("W1", [IN_DIM, HID], BF16, kind="ExternalInput")
    W2p_d = nc.dram_tensor("W2p", [128, OUTP], BF16, kind="ExternalInput")
    b1t_d = nc.dram_tensor("b1t", [128, HID], F32, kind="ExternalInput")
    b2t_d = nc.dram_tensor("b2t", [128, OUTP], F32, kind="ExternalInput")
    out_d = nc.dram_tensor("out", [SHARD, OUT], F32, kind="ExternalOutput")

    t1l = nc.dram_tensor("t1l", [ROWS, HID], F32, kind="Internal")
    t1f = nc.dram_tensor("t1f", [ROWS * NCORES, HID], F32, kind="Internal",
                         addr_space="Shared")
    HW2 = 128  # h~ stored 128-wide (DMA-transpose needs free dim %128)
    hl = nc.dram_tensor("hl", [HPAD, HW2], BF16, kind="Internal")
    t2l = nc.dram_tensor("t2l", [ROWS, OUTP], F32, kind="Internal")
    t2f = nc.dram_tensor("t2f", [ROWS * NCORES, OUTP], F32, kind="Internal",
                         addr_space="Shared")

    rg = [list(range(NCORES))]

    with tile.TileContext(nc) as tc:
        with (
            tc.tile_pool(name="meta", bufs=1) as meta,
            tc.tile_pool(name="mm", bufs=3) as mm,
            tc.tile_pool(name="ps", bufs=4, space="PSUM") as ps,
            tc.tile_pool(name="gat", bufs=3) as gat,
            tc.tile_pool(name="red", bufs=3) as red,
            tc.tile_pool(name="epi", bufs=3) as epi,
        ):
            # ---- resident metadata/constants ----
            xT_sb = meta.tile([IN_DIM, SHARD_PAD], BF16, tag="bigT")
            nc.sync.dma_start(out=xT_sb[:], in_=xT[:])
            gidx_sb = meta.tile([128, NCH], I32)
            nc.sync.dma_start(out=gidx_sb[:], in_=gidx_d[:])
            disw_sb = meta.tile([128, NWIN], F32)
            nc.sync.dma_start(out=disw_sb[:], in_=disw_d[:])
            scat_sb = meta.tile([128, NWIN], I32)
            nc.sync.dma_start(out=scat_sb[:], in_=scat_d[:])
            disid_sb = meta.tile([128, NT], F32)
            nc.sync.dma_start(out=disid_sb[:], in_=disid_d[:])
            W1_sb = meta.tile([IN_DIM, HID], BF16)
            nc.sync.dma_start(out=W1_sb[:], in_=W1_d[:])
            W2p_sb = meta.tile([128, OUTP], BF16)
            nc.sync.dma_start(out=W2p_sb[:], in_=W2p_d[:])
            b1t_sb = meta.tile([128, HID], F32)
            nc.sync.dma_start(out=b1t_sb[:], in_=b1t_d[:])
            b2t_sb = meta.tile([128, OUTP], F32)
            nc.sync.dma_start(out=b2t_sb[:], in_=b2t_d[:])
            zero_sb = meta.tile([128, max(HID, OUTP)], F32)
            nc.vector.memset(zero_sb[:], 0.0)
            zero_bf = meta.tile([128, HW2], BF16)
            nc.vector.memset(zero_bf[:], 0.0)

            # zero rows: table zero row + h_local ghost tail
            nc.sync.dma_start(out=t1l[SHARD : SHARD + 1, :], in_=zero_sb[:1, :HID])
            nc.sync.dma_start(out=t2l[SHARD : SHARD + 1, :], in_=zero_sb[:1, :OUTP])
            if HPAD > SHARD:
                nc.sync.dma_start(
                    out=hl[SHARD:HPAD, :], in_=zero_bf[: HPAD - SHARD, :]
                )

            # ---- phase 1: T1 = dis * (x @ W1) ----
            for t in range(NT):
                p1 = ps.tile([128, HID], F32, tag="p1")
                nc.tensor.matmul(
                    out=p1[:],
                    lhsT=xT_sb[:, t * 128 : (t + 1) * 128],
                    rhs=W1_sb[:],
                    start=True,
                    stop=True,
                )
                st = mm.tile([128, HID], F32, tag="st1")
                nc.vector.tensor_scalar(
                    out=st[:], in0=p1[:], scalar1=disid_sb[:, t : t + 1],
                    scalar2=None, op0=mybir.AluOpType.mult,
                )
                hi = min((t + 1) * 128, SHARD) - t * 128
                nc.sync.dma_start(
                    out=t1l[t * 128 : t * 128 + hi, :], in_=st[:hi, :]
                )

            nc.gpsimd.collective_compute(
                "AllGather", mybir.AluOpType.bypass, replica_groups=rg,
                ins=[t1l[:]], outs=[t1f[:]],
            )

            # ---- phase 2: layer-1 aggregation ----
            for w in range(NWIN):
                Kw = K[w]
                g = gat.tile([128, Kw * HID], F32, tag="g1")
                for c in range(Kw):
                    nc.gpsimd.indirect_dma_start(
                        out=g[:, c * HID : (c + 1) * HID],
                        out_offset=None,
                        in_=t1f[:],
                        in_offset=bass.IndirectOffsetOnAxis(
                            ap=gidx_sb[:, coff[w] + c : coff[w] + c + 1], axis=0
                        ),
                    )
                acc = _tree_reduce(nc, red, g, Kw, HID)
                dw = disw_sb[:, w : w + 1]
                t_ = epi.tile([128, HID], F32, tag="t1e")
                nc.vector.tensor_scalar(
                    out=t_[:], in0=acc[:], scalar1=dw, scalar2=None,
                    op0=mybir.AluOpType.mult,
                )
                nc.vector.tensor_add(t_[:], t_[:], b1t_sb[:])
                hb = epi.tile([128, HW2], BF16, tag="hbe")
                nc.vector.memset(hb[:, HID:], 0.0)
                nc.vector.tensor_scalar(
                    out=hb[:, :HID], in0=t_[:], scalar1=0.0, scalar2=dw,
                    op0=mybir.AluOpType.max, op1=mybir.AluOpType.mult,
                )
                nc.gpsimd.indirect_dma_start(
                    out=hl[:],
                    out_offset=bass.IndirectOffsetOnAxis(
                        ap=scat_sb[:, w : w + 1], axis=0
                    ),
                    in_=hb[:],
                    in_offset=None,
                    bounds_check=SHARD - 1,
                    oob_is_err=False,
                )

            # ---- phase 3: T2 = h~ @ W2 (dis already folded into h~) ----
            hT_sb = meta.tile([HW2, HPAD], BF16, tag="bigT")
            nc.sync.dma_start(out=hT_sb[:], in_=hl[:], transpose=True)
            for t in range(NT):
                p2 = ps.tile([128, OUTP], F32, tag="p2")
                nc.tensor.matmul(
                    out=p2[:],
                    lhsT=hT_sb[:, t * 128 : (t + 1) * 128],
                    rhs=W2p_sb[:],
                    start=True,
                    stop=True,
                )
                st2 = mm.tile([128, OUTP], F32, tag="st2")
                nc.vector.tensor_copy(st2[:], p2[:])
                hi = min((t + 1) * 128, SHARD) - t * 128
                nc.sync.dma_start(
                    out=t2l[t * 128 : t * 128 + hi, :], in_=st2[:hi, :]
                )

            nc.gpsimd.collective_compute(
                "AllGather", mybir.AluOpType.bypass, replica_groups=rg,
                ins=[t2l[:]], outs=[t2f[:]],
            )

            # ---- phase 4: layer-2 aggregation -> output ----
            for w in range(NWIN):
                Kw = K[w]
                g = gat.tile([128, Kw * OUTP], F32, tag="g2")
                for c in range(Kw):
                    nc.gpsimd.indirect_dma_start(
                        out=g[:, c * OUTP : (c + 1) * OUTP],
                        out_offset=None,
                        in_=t2f[:],
                        in_offset=bass.IndirectOffsetOnAxis(
                            ap=gidx_sb[:, coff[w] + c : coff[w] + c + 1], axis=0
                        ),
                    )
                acc = _tree_reduce(nc, red, g, Kw, OUTP)
                dw = disw_sb[:, w : w + 1]
                t_ = epi.tile([128, OUTP], F32, tag="t2e")
                nc.vector.tensor_scalar(
                    out=t_[:], in0=acc[:], scalar1=dw, scalar2=None,
                    op0=mybir.AluOpType.mult,
                )
                ot = epi.tile([128, OUT], F32, tag="ote")
                nc.vector.tensor_add(ot[:], t_[:, :OUT], b2t_sb[:, :OUT])
                nc.gpsimd.indirect_dma_start(
                    out=out_d[:],
                    out_offset=bass.IndirectOffsetOnAxis(
                        ap=scat_sb[:, w : w + 1], axis=0
                    ),
                    in_=ot[:],
                    in_offset=None,
                    bounds_check=SHARD - 1,
                    oob_is_err=False,
                )

    nc.compile()
    return nc


def kernel(x, edge_index, W1, b1, W2, b2):
    x = np.asarray(x)
    edge_index = np.asarray(edge_index)
    W1 = np.asarray(W1)
    b1 = np.asarray(b1)
    W2 = np.asarray(W2)
    b2 = np.asarray(b2)
    in_maps, K, coff, dims = _prep(x, edge_index, W1, b1, W2, b2)
    nc = _build(K, coff, dims)
    import time as _time

    # correctness path (first call pays PJRT/NeuronCC jit compile)
    res = bass_utils.run_bass_kernel_spmd(
        nc, in_maps, core_ids=list(range(NCORES))
    )
    global LAST_EXEC_NS
    try:
        LAST_EXEC_NS = _timed_device_resident(nc, in_maps)
    except Exception:
        t0 = _time.perf_counter()
        bass_utils.run_bass_kernel_spmd(nc, in_maps, core_ids=list(range(NCORES)))
        LAST_EXEC_NS = int((_time.perf_counter() - t0) * 1e9)
    out = np.concatenate([res.results[c]["out"] for c in range(NCORES)], axis=0)
    return out.astype(np.float32)


LAST_EXEC_NS = -1


def _timed_device_resident(nc, in_maps):
    """Time NEFF execution with inputs pre-placed on the 8 devices.

    Mirrors bass2jax.run_bass_via_pjrt's shard_map wiring but device_puts the
    global operands once, so the timed call measures execution + dispatch
    rather than per-call host<->device transfer. Measurement only — kernel
    outputs come from the standard path.
    """
    import time as _time

    import jax
    import concourse.mybir as mb
    from concourse import bass2jax
    from jax.experimental.shard_map import shard_map
    from jax.sharding import Mesh, NamedSharding, PartitionSpec

    in_names, out_names, out_avals, zero_outs = [], [], [], []
    for alloc in nc.m.functions[0].allocations:
        if not isinstance(alloc, mb.MemoryLocationSet):
            continue
        name = alloc.memorylocations[0].name
        if alloc.kind == "ExternalInput":
            in_names.append(name)
        elif alloc.kind == "ExternalOutput":
            out_names.append(name)
            shape = tuple(alloc.tensor_shape)
            dtype = mb.dt.np(alloc.dtype)
            out_avals.append(jax.core.ShapedArray(shape, dtype))
            zero_outs.append(np.zeros(shape, dtype))
    n_params = len(in_names)
    all_names = in_names + out_names

    def _body(*args):
        return tuple(
            bass2jax._bass_exec_p.bind(
                *args,
                out_avals=tuple(out_avals),
                in_names=tuple(all_names),
                out_names=tuple(out_names),
                lowering_input_output_aliases=(),
                sim_require_finite=True,
                sim_require_nnan=True,
                nc=nc,
            )
        )

    devices = jax.devices()[:NCORES]
    mesh = Mesh(np.asarray(devices), ("core",))
    spec = PartitionSpec("core")
    f = jax.jit(
        shard_map(
            _body,
            mesh=mesh,
            in_specs=(spec,) * (n_params + len(out_names)),
            out_specs=(spec,) * len(out_names),
            check_rep=False,
        ),
        keep_unused=True,
    )
    sh = NamedSharding(mesh, spec)
    ops = [
        jax.device_put(
            np.concatenate([np.asarray(m[nm]) for m in in_maps], axis=0), sh
        )
        for nm in in_names
    ] + [
        jax.device_put(np.concatenate([z] * NCORES, axis=0), sh)
        for z in zero_outs
    ]
    outs = f(*ops)  # warm-up / compile
    jax.block_until_ready(outs)
    best = None
    for _ in range(2):
        t0 = _time.perf_counter()
        outs = f(*ops)
        jax.block_until_ready(outs)
        dt = _time.perf_counter() - t0
        best = dt if best is None or dt < best else best
    return int(best * 1e9)


# revision 15
# speedup vs baseline: 1.0427x; 1.0427x over previous
"""2-layer GCN (PyG GCNConv semantics) on 8 Trainium2 NeuronCores.

Strategy (per the node-sharding hint):
  - Nodes are sharded contiguously across the 8 cores (dst-ownership).
  - Layer tables T1 = dis*(x@W1)  [N,64] f32 and T2 = (dis*relu(out1))@W2
    [N,48] f32 are computed shard-locally and AllGather'd so every core can
    gather any source row.
  - Per core, edges are grouped by destination into fixed-K windows of 128
    degree-sorted nodes; messages are fetched with 128-row indirect DMA
    gathers (one index per partition), summed with a tensor_tensor add tree,
    scaled by dis[dst], biased, (relu'd), and scattered back to node-id rows
    with an indirect DMA scatter.
  - dis[src] is folded into the tables; dis[dst] is a per-partition scalar.

kernel(**inputs) takes the FULL inputs and returns the FULL [N,40] output.
"""

import numpy as np
import ml_dtypes

import concourse.bass as bass
import concourse.bacc as bacc
import concourse.tile as tile
import concourse.mybir as mybir
from concourse import bass_utils

F32 = mybir.dt.float32
BF16 = mybir.dt.bfloat16
I32 = mybir.dt.int32

NCORES = 8
GHOST = 1 << 20  # scatter index sentinel, skipped via bounds_check


def _round_up(x, m):
    return ((x + m - 1) // m) * m


def _prep(x, edge_index, W1, b1, W2, b2):
    """Host-side graph partitioning + metadata packing (numpy only)."""
    N, IN_DIM = x.shape
    HID = W1.shape[1]
    OUT = W2.shape[1]
    OUTP = _round_up(OUT, 8)  # padded table-2 width (48 for OUT=40)
    assert N % NCORES == 0
    SHARD = N // NCORES  # nodes per core
    ROWS = SHARD + 1  # per-shard table rows incl zero row
    NT = _round_up(SHARD, 128) // 128  # 128-node tiles per shard
    SHARD_PAD = NT * 128

    src = edge_index[0].astype(np.int64)
    dst = edge_index[1].astype(np.int64)
    # self-loops
    loops = np.arange(N, dtype=np.int64)
    src = np.concatenate([src, loops])
    dst = np.concatenate([dst, loops])
    deg = np.bincount(dst, minlength=N).astype(np.float64)  # >=1 (self-loops)
    dis = (1.0 / np.sqrt(deg)).astype(np.float32)

    # global table row of node g (tables have a zero row per shard)
    def grow(g):
        return (g // SHARD) * ROWS + (g % SHARD)

    core_of = (dst // SHARD).astype(np.int64)

    # pass 1: per-core degree-sorted windows -> global K schedule
    orders = []
    degl_list = []
    for c in range(NCORES):
        m = core_of == c
        dl = (dst[m] - c * SHARD).astype(np.int64)
        degl = np.bincount(dl, minlength=SHARD)
        order = np.argsort(-degl, kind="stable").astype(np.int64)  # desc degree
        orders.append(order)
        degl_list.append(degl)
    NWIN = NT
    K = np.zeros(NWIN, dtype=np.int64)
    for c in range(NCORES):
        degl = degl_list[c]
        order = orders[c]
        for w in range(NWIN):
            nodes = order[w * 128 : (w + 1) * 128]
            if len(nodes):
                K[w] = max(K[w], degl[nodes].max() if len(nodes) else 0)
    K = np.maximum(((K + 1) // 2) * 2, 2)
    coff = np.concatenate([[0], np.cumsum(K)]).astype(np.int64)
    NCH = int(coff[-1])

    in_maps = []
    for c in range(NCORES):
        m = core_of == c
        s_c = src[m]
        d_c = dst[m]
        dl = (d_c - c * SHARD).astype(np.int64)
        order = orders[c]
        inv = np.empty(SHARD, dtype=np.int64)
        inv[order] = np.arange(SHARD)
        pos = inv[dl]  # degree-sorted position of each edge's dst
        o2 = np.argsort(pos, kind="stable")
        pos_s = pos[o2]
        src_s = s_c[o2]
        first = np.searchsorted(pos_s, pos_s, side="left")
        slot = np.arange(len(pos_s)) - first
        lane = pos_s % 128
        win = pos_s // 128
        col = coff[win] + slot
        ZROW = SHARD  # shard 0's zero row (global table row = SHARD)
        gidx = np.full((128, NCH), ZROW, dtype=np.int32)
        gidx[lane, col] = grow(src_s).astype(np.int32)

        # window metadata in degree-sorted order
        node_of = np.full((128, NWIN), -1, dtype=np.int64)
        for w in range(NWIN):
            nodes = order[w * 128 : min((w + 1) * 128, SHARD)]
            node_of[: len(nodes), w] = nodes
        real = node_of >= 0
        disw = np.zeros((128, NWIN), dtype=np.float32)
        disw[real] = dis[c * SHARD + node_of[real]]
        scat = np.full((128, NWIN), GHOST, dtype=np.int32)
        scat[real] = node_of[real].astype(np.int32)

        # id-order dis for phase 1/3 epilogues (padded tail -> 0)
        disid = np.zeros((128, NT), dtype=np.float32)
        ids = np.arange(SHARD_PAD).reshape(NT, 128).T
        okm = ids < SHARD
        disid[okm] = dis[c * SHARD + ids[okm]]

        xT = np.zeros((IN_DIM, SHARD_PAD), dtype=ml_dtypes.bfloat16)
        xT[:, :SHARD] = x[c * SHARD : (c + 1) * SHARD].T.astype(
            ml_dtypes.bfloat16
        )

        W2p = np.zeros((128, OUTP), dtype=ml_dtypes.bfloat16)
        W2p[:HID, :OUT] = W2.astype(ml_dtypes.bfloat16)

        in_maps.append(
            {
                "xT": xT,
                "gidx": gidx,
                "disw": disw,
                "scat": scat,
                "disid": disid,
                "W1": W1.astype(ml_dtypes.bfloat16),
                "W2p": W2p,
                "b1t": np.tile(np.asarray(b1, np.float32)[None, :], (128, 1)),
                "b2t": np.tile(
                    np.pad(np.asarray(b2, np.float32), (0, OUTP - OUT))[None, :],
                    (128, 1),
                ),
            }
        )

    dims = dict(
        N=N, IN_DIM=IN_DIM, HID=HID, OUT=OUT, OUTP=OUTP, SHARD=SHARD, ROWS=ROWS,
        NT=NT, SHARD_PAD=SHARD_PAD, NWIN=NWIN, NCH=NCH,
    )
    return in_maps, [int(k) for k in K], [int(v) for v in coff], dims


def _tree_reduce(nc, pool, g, K, F):
    """Sum g's [128, K, F] f32 chunks into a [128, F] tile."""
    cur = g
    n = K
    lvl = 0
    while n % 2 == 0 and n > 2:
        h = n // 2
        dst = pool.tile([128, h * F], F32, tag=f"lvl{lvl}")
        nc.vector.tensor_add(dst[:], cur[:, : h * F], cur[:, h * F : 2 * h * F])
        cur = dst
        n = h
        lvl += 1
    acc = pool.tile([128, F], F32, tag="acc")
    nc.vector.tensor_add(acc[:], cur[:, :F], cur[:, F : 2 * F])
    for i in range(2, n):
        nc.vector.tensor_add(acc[:], acc[:], cur[:, i * F : (i + 1) * F])
    return acc


def _build(K, coff, d):
    N, HID, OUTP, OUT = d["N"], d["HID"], d["OUTP"], d["OUT"]
    IN_DIM, SHARD, ROWS, NT = d["IN_DIM"], d["SHARD"], d["ROWS"], d["NT"]
    SHARD_PAD, NWIN, NCH = d["SHARD_PAD"], d["NWIN"], d["NCH"]
    HPAD = SHARD_PAD  # h_local rows (multiple of 128 for DMA transpose)

    nc = bacc.Bacc("TRN2", target_bir_lowering=False, debug=False,
                   num_devices=NCORES)
    xT = nc.dram_tensor("xT", [IN_DIM, SHARD_PAD], BF16, kind="ExternalInput")
    gidx_d = nc.dram_tensor("gidx", [128, NCH], I32, kind="ExternalInput")
    disw_d = nc.dram_tensor("disw", [128, NWIN], F32, kind="ExternalInput")
    scat_d = nc.dram_tensor("scat", [128, NWIN], I32, kind="ExternalInput")
    disid_d = nc.dram_tensor("disid", [128, NT], F32, kind="ExternalInput")
    W1_d = nc.dram_tensor("W1", [IN_DIM, HID], BF16, kind="ExternalInput")
    W2p_d = nc.dram_tensor("W2p", [128, OUTP], BF16, kind="ExternalInput")
    b1t_d = nc.dram_tensor("b1t", [128, HID], F32, kind="ExternalInput")
    b2t_d = nc.dram_tensor("b2t", [128, OUTP], F32, kind="ExternalInput")
    out_d = nc.dram_tensor("out", [SHARD, OUT], F32, kind="ExternalOutput")

    t1l = nc.dram_tensor("t1l", [ROWS, HID], F32, kind="Internal")
    t1f = nc.dram_tensor("t1f", [ROWS * NCORES, HID], F32, kind="Internal",
                         addr_space="Shared")
    HW2 = 128  # h~ stored 128-wide (DMA-transpose needs free dim %128)
    hl = nc.dram_tensor("hl", [HPAD, HW2], BF16, kind="Internal")
    t2l = nc.dram_tensor("t2l", [ROWS, OUTP], F32, kind="Internal")
    t2f = nc.dram_tensor("t2f", [ROWS * NCORES, OUTP], F32, kind="Internal",
                         addr_space="Shared")

    rg = [list(range(NCORES))]

    with tile.TileContext(nc) as tc:
        with (
            tc.tile_pool(name="meta", bufs=1) as meta,
            tc.tile_pool(name="mm", bufs=3) as mm,
            tc.tile_pool(name="ps", bufs=4, space="PSUM") as ps,
            tc.tile_pool(name="gat", bufs=2) as gat,
            tc.tile_pool(name="red", bufs=2) as red,
            tc.tile_pool(name="epi", bufs=3) as epi,
        ):
            # ---- resident metadata/constants ----
            xT_sb = meta.tile([IN_DIM, SHARD_PAD], BF16, tag="bigT")
            nc.sync.dma_start(out=xT_sb[:], in_=xT[:])
            gidx_sb = meta.tile([128, NCH], I32)
            nc.sync.dma_start(out=gidx_sb[:], in_=gidx_d[:])
            disw_sb = meta.tile([128, NWIN], F32)
            nc.sync.dma_start(out=disw_sb[:], in_=disw_d[:])
            scat_sb = meta.tile([128, NWIN], I32)
            nc.sync.dma_start(out=scat_sb[:], in_=scat_d[:])
            disid_sb = meta.tile([128, NT], F32)
            nc.sync.dma_start(out=disid_sb[:], in_=disid_d[:])
            W1_sb = meta.tile([IN_DIM, HID], BF16)
            nc.sync.dma_start(out=W1_sb[:], in_=W1_d[:])
            W2p_sb = meta.tile([128, OUTP], BF16)
            nc.sync.dma_start(out=W2p_sb[:], in_=W2p_d[:])
            b1t_sb = meta.tile([128, HID], F32)
            nc.sync.dma_start(out=b1t_sb[:], in_=b1t_d[:])
            b2t_sb = meta.tile([128, OUTP], F32)
            nc.sync.dma_start(out=b2t_sb[:], in_=b2t_d[:])
            zero_sb = meta.tile([128, max(HID, OUTP)], F32)
            nc.vector.memset(zero_sb[:], 0.0)
            zero_bf = meta.tile([128, HW2], BF16)
            nc.vector.memset(zero_bf[:], 0.0)

            # zero rows: table zero row + h_local ghost tail
            nc.sync.dma_start(out=t1l[SHARD : SHARD + 1, :], in_=zero_sb[:1, :HID])
            nc.sync.dma_start(out=t2l[SHARD : SHARD + 1, :], in_=zero_sb[:1, :OUTP])
            if HPAD > SHARD:
                nc.sync.dma_start(
                    out=hl[SHARD:HPAD, :], in_=zero_bf[: HPAD - SHARD, :]
                )

            # ---- phase 1: T1 = dis * (x @ W1) ----
            for t in range(NT):
                p1 = ps.tile([128, HID], F32, tag="p1")
                nc.tensor.matmul(
                    out=p1[:],
                    lhsT=xT_sb[:, t * 128 : (t + 1) * 128],
                    rhs=W1_sb[:],
                    start=True,
                    stop=True,
                )
                st = mm.tile([128, HID], F32, tag="st1")
                nc.vector.tensor_scalar(
                    out=st[:], in0=p1[:], scalar1=disid_sb[:, t : t + 1],
                    scalar2=None, op0=mybir.AluOpType.mult,
                )
                hi = min((t + 1) * 128, SHARD) - t * 128
                nc.sync.dma_start(
                    out=t1l[t * 128 : t * 128 + hi, :], in_=st[:hi, :]
                )

            nc.gpsimd.collective_compute(
                "AllGather", mybir.AluOpType.bypass, replica_groups=rg,
                ins=[t1l[:]], outs=[t1f[:]],
            )

            # ---- phase 2: layer-1 aggregation ----
            for w in range(NWIN):
                Kw = K[w]
                g = gat.tile([128, Kw * HID], F32, tag="g1")
                for c in range(Kw):
                    nc.gpsimd.indirect_dma_start(
                        out=g[:, c * HID : (c + 1) * HID],
                        out_offset=None,
                        in_=t1f[:],
                        in_offset=bass.IndirectOffsetOnAxis(
                            ap=gidx_sb[:, coff[w] + c : coff[w] + c + 1], axis=0
                        ),
                    )
                acc = _tree_reduce(nc, red, g, Kw, HID)
                dw = disw_sb[:, w : w + 1]
                t_ = epi.tile([128, HID], F32, tag="t1e")
                nc.vector.tensor_scalar(
                    out=t_[:], in0=acc[:], scalar1=dw, scalar2=None,
                    op0=mybir.AluOpType.mult,
                )
                nc.vector.tensor_add(t_[:], t_[:], b1t_sb[:])
                hb = epi.tile([128, HW2], BF16, tag="hbe")
                nc.vector.memset(hb[:, HID:], 0.0)
                nc.vector.tensor_scalar(
                    out=hb[:, :HID], in0=t_[:], scalar1=0.0, scalar2=dw,
                    op0=mybir.AluOpType.max, op1=mybir.AluOpType.mult,
                )
                nc.gpsimd.indirect_dma_start(
                    out=hl[:],
                    out_offset=bass.IndirectOffsetOnAxis(
                        ap=scat_sb[:, w : w + 1], axis=0
                    ),
                    in_=hb[:],
                    in_offset=None,
                    bounds_check=SHARD - 1,
                    oob_is_err=False,
                )

            # ---- phase 3: T2 = h~ @ W2 (dis already folded into h~) ----
            hT_sb = meta.tile([HW2, HPAD], BF16, tag="bigT")
            nc.sync.dma_start(out=hT_sb[:], in_=hl[:], transpose=True)
            for t in range(NT):
                p2 = ps.tile([128, OUTP], F32, tag="p2")
                nc.tensor.matmul(
                    out=p2[:],
                    lhsT=hT_sb[:, t * 128 : (t + 1) * 128],
                    rhs=W2p_sb[:],
                    start=True,
                    stop=True,
                )
                st2 = mm.tile([128, OUTP], F32, tag="st2")
                nc.vector.tensor_copy(st2[:], p2[:])
                hi = min((t + 1) * 128, SHARD) - t * 128
                nc.sync.dma_start(
                    out=t2l[t * 128 : t * 128 + hi, :], in_=st2[:hi, :]
                )

            nc.gpsimd.collective_compute(
                "AllGather", mybir.AluOpType.bypass, replica_groups=rg,
                ins=[t2l[:]], outs=[t2f[:]],
            )

            # ---- phase 4: layer-2 aggregation -> output ----
            for w in range(NWIN):
                Kw = K[w]
                g = gat.tile([128, Kw * OUTP], F32, tag="g2")
                for c in range(Kw):
                    nc.gpsimd.indirect_dma_start(
                        out=g[:, c * OUTP : (c + 1) * OUTP],
                        out_offset=None,
                        in_=t2f[:],
                        in_offset=bass.IndirectOffsetOnAxis(
                            ap=gidx_sb[:, coff[w] + c : coff[w] + c + 1], axis=0
                        ),
                    )
                acc = _tree_reduce(nc, red, g, Kw, OUTP)
                dw = disw_sb[:, w : w + 1]
                t_ = epi.tile([128, OUTP], F32, tag="t2e")
                nc.vector.tensor_scalar(
                    out=t_[:], in0=acc[:], scalar1=dw, scalar2=None,
                    op0=mybir.AluOpType.mult,
                )
                ot = epi.tile([128, OUT], F32, tag="ote")
                nc.vector.tensor_add(ot[:], t_[:, :OUT], b2t_sb[:, :OUT])
                nc.gpsimd.indirect_dma_start(
                    out=out_d[:],
                    out_offset=bass.IndirectOffsetOnAxis(
                        ap=scat_sb[:, w : w + 1], axis=0
                    ),
                    in_=ot[:],
                    in_offset=None,
                    bounds_check=SHARD - 1,
                    oob_is_err=False,
                )

    nc.compile()
    return nc


def kernel(x, edge_index, W1, b1, W2, b2):
    x = np.asarray(x)
    edge_index = np.asarray(edge_index)
    W1 = np.asarray(W1)
    b1 = np.asarray(b1)
    W2 = np.asarray(W2)
    b2 = np.asarray(b2)
    in_maps, K, coff, dims = _prep(x, edge_index, W1, b1, W2, b2)
    nc = _build(K, coff, dims)
    import time as _time

    # correctness path (first call pays PJRT/NeuronCC jit compile)
    res = bass_utils.run_bass_kernel_spmd(
        nc, in_maps, core_ids=list(range(NCORES))
    )
    global LAST_EXEC_NS
    try:
        LAST_EXEC_NS = _timed_device_resident(nc, in_maps)
    except Exception:
        t0 = _time.perf_counter()
        bass_utils.run_bass_kernel_spmd(nc, in_maps, core_ids=list(range(NCORES)))
        LAST_EXEC_NS = int((_time.perf_counter() - t0) * 1e9)
    out = np.concatenate([res.results[c]["out"] for c in range(NCORES)], axis=0)
    return out.astype(np.float32)


LAST_EXEC_NS = -1


def _timed_device_resident(nc, in_maps):
    """Time NEFF execution with inputs pre-placed on the 8 devices.

    Mirrors bass2jax.run_bass_via_pjrt's shard_map wiring but device_puts the
    global operands once, so the timed call measures execution + dispatch
    rather than per-call host<->device transfer. Measurement only — kernel
    outputs come from the standard path.
    """
    import time as _time

    import jax
    import concourse.mybir as mb
    from concourse import bass2jax
    from jax.experimental.shard_map import shard_map
    from jax.sharding import Mesh, NamedSharding, PartitionSpec

    in_names, out_names, out_avals, zero_outs = [], [], [], []
    for alloc in nc.m.functions[0].allocations:
        if not isinstance(alloc, mb.MemoryLocationSet):
            continue
        name = alloc.memorylocations[0].name
        if alloc.kind == "ExternalInput":
            in_names.append(name)
        elif alloc.kind == "ExternalOutput":
            out_names.append(name)
            shape = tuple(alloc.tensor_shape)
            dtype = mb.dt.np(alloc.dtype)
            out_avals.append(jax.core.ShapedArray(shape, dtype))
            zero_outs.append(np.zeros(shape, dtype))
    n_params = len(in_names)
    all_names = in_names + out_names

    def _body(*args):
        return tuple(
            bass2jax._bass_exec_p.bind(
                *args,
                out_avals=tuple(out_avals),
                in_names=tuple(all_names),
                out_names=tuple(out_names),
                lowering_input_output_aliases=(),
                sim_require_finite=True,
                sim_require_nnan=True,
                nc=nc,
            )
        )

    devices = jax.devices()[:NCORES]
    mesh = Mesh(np.asarray(devices), ("core",))
    spec = PartitionSpec("core")
    f = jax.jit(
        shard_map(
            _body,
            mesh=mesh,
            in_specs=(spec,) * (n_params + len(out_names)),
            out_specs=(spec,) * len(out_names),
            check_rep=False,
        ),
        keep_unused=True,
    )
    sh = NamedSharding(mesh, spec)
    ops = [
        jax.device_put(
            np.concatenate([np.asarray(m[nm]) for m in in_maps], axis=0), sh
        )
        for nm in in_names
    ] + [
        jax.device_put(np.concatenate([z] * NCORES, axis=0), sh)
        for z in zero_outs
    ]
    outs = f(*ops)  # warm-up / compile
    jax.block_until_ready(outs)
    best = None
    for _ in range(2):
        t0 = _time.perf_counter()
        outs = f(*ops)
        jax.block_until_ready(outs)
        dt = _time.perf_counter() - t0
        best = dt if best is None or dt < best else best
    return int(best * 1e9)
